# revision 16
# baseline (speedup 1.0000x reference)
"""DirVGAE (GATv2 x2 -> Dirichlet z -> sigmoid(z z^T)) on 8 trn2 NeuronCores.

Sharding: dst-nodes row-sharded across 8 cores (1536 rows each); GAT weights
replicated; edge lists partitioned by destination core; dense sigmoid(z z^T)
row-sharded the same way.

GAT layer on device (transposed layout, host-staged gathers):
  The slot->src map is data-independent, so the source-feature "halo
  exchange" is staged on host: ship x^T[:, src(slot)] (f16) per core; the
  device applies Wl via one stationary matmul (gather commutes with the
  linear transform), adds the xr broadcast, does LeakyReLU + attention dot
  (per-tile PE matmuls), segment softmax over per-partition windows, and
  aggregates via alpha-replicate (per-tile transpose-LDW) + windowed reduce.
Adjacency: bf16 z @ z^T on PE + sigmoid evac (f16 out).
Gamma sampling: jax CPU (bit-matches the reference sampler).
"""
import sys, os, time
sys.path.insert(0, "/opt/trn_rl_repo")
import numpy as np
from contextlib import ExitStack

import concourse.bass as bass
import concourse.tile as tile
from concourse import bacc, mybir
from concourse.bass_utils import run_bass_kernel_spmd

F32 = mybir.dt.float32
F16 = mybir.dt.float16
BF16 = mybir.dt.bfloat16

N, E = 12288, 393216
IN, HID, OUT = 128, 128, 64
NEG_SLOPE = 0.2
NC = 8
NLOC = N // NC
ROWS = NLOC // 128

HOST_GAT = os.environ.get("HOST_GAT", "0") == "1"

_cache = {}
LAST_HW_NS = 0
_neff_ns = {}


def _sim_ns(key, nc):
    """Cost-model estimate of one NEFF execution (per core), cached."""
    if key not in _neff_ns:
        try:
            from concourse.timeline_sim import TimelineSim
            _neff_ns[key] = int(TimelineSim(nc).simulate())
        except Exception:
            _neff_ns[key] = 0
    return _neff_ns[key]


def _edge_layout(edge_index):
    """Per-core slot layout: nodes ranked by padded degree, rows of 128,
    common D per row; slot s = j*128 + p holds edge d=(j-W_k) of node
    rank k*128+p. Returns per-core dicts + common D_row."""
    src = edge_index[0].astype(np.int64)
    dst = edge_index[1].astype(np.int64)
    cores = []
    for ci in range(NC):
        r0 = ci * NLOC
        sel = (dst >= r0) & (dst < r0 + NLOC)
        s_i = src[sel]
        d_i = dst[sel] - r0
        order = np.argsort(d_i, kind="stable")
        s_i = s_i[order]
        d_i = d_i[order]
        deg = np.bincount(d_i, minlength=NLOC)
        starts = np.concatenate([[0], np.cumsum(deg)])
        Dpad = np.maximum(4, ((deg + 3) // 4) * 4)
        rank_of = np.argsort(Dpad, kind="stable")
        D_row = np.array([Dpad[rank_of[k*128:(k+1)*128]].max() for k in range(ROWS)])
        cores.append(dict(r0=r0, s_i=s_i, deg=deg, starts=starts,
                          rank_of=rank_of, D_row=D_row))
    Dc = np.stack([c["D_row"] for c in cores]).max(axis=0)
    W = np.concatenate([[0], np.cumsum(Dc)])
    G = int(W[-1])
    for lay in cores:
        deg, starts, rank_of, s_i = (lay["deg"], lay["starts"],
                                     lay["rank_of"], lay["s_i"])
        gsrc = np.zeros(G * 128, dtype=np.int32)
        mask = np.zeros((128, G), dtype=np.float16)
        node_of = np.zeros(NLOC, dtype=np.int64)
        for k in range(ROWS):
            Dk = int(Dc[k])
            Wk = int(W[k])
            nodes = rank_of[k * 128:(k + 1) * 128]
            node_of[k * 128:(k + 1) * 128] = nodes + lay["r0"]
            for p in range(128):
                n = nodes[p]
                d = int(deg[n])
                e0 = int(starts[n])
                sl = s_i[e0:e0 + d]
                jj = (Wk + np.arange(d)) * 128 + p
                gsrc[jj] = sl
                mask[p, Wk:Wk + d] = 1.0
        lay["gsrc"] = gsrc
        lay["mask"] = mask
        lay["node_of"] = node_of
    return cores, Dc, W, G


def _gat_host(cores, G, W, Dc, xl, xr, att, bias_out, relu):
    """Host fallback of the device layer (same math, f32)."""
    C = xl.shape[1]
    out = np.zeros((N, C), dtype=np.float32)
    for lay in cores:
        xg = xl[lay["gsrc"], :]                       # [G*128, C]
        xrv = xr[lay["node_of"], :]                   # [1536, C]
        mask = lay["mask"].astype(np.float32)
        e = np.zeros((128, G), dtype=np.float32)
        for k in range(ROWS):
            Dk, Wk = int(Dc[k]), int(W[k])
            blk = xg[Wk*128:(Wk+Dk)*128].reshape(Dk, 128, C)
            s = blk + xrv[k*128:(k+1)*128][None, :, :]
            h = np.where(s > 0, s, NEG_SLOPE * s)
            e[:, Wk:Wk+Dk] = (h @ att).T
        a = np.exp(e) * mask
        for k in range(ROWS):
            Dk, Wk = int(Dc[k]), int(W[k])
            aw = a[:, Wk:Wk+Dk]
            al = aw / (aw.sum(1, keepdims=True) + 1e-16)
            blk = xg[Wk*128:(Wk+Dk)*128].reshape(Dk, 128, C)
            o = (al.T[:, :, None] * blk).sum(0) + bias_out
            out[lay["node_of"][k*128:(k+1)*128]] = np.maximum(o, 0) if relu else o
    return out


def _build_gat_neff(Dc, C):
    nc = bacc.Bacc("TRN2", target_bir_lowering=False, debug=False,
                   num_devices=NC)
    W = np.concatenate([[0], np.cumsum(Dc)])
    G = int(W[-1])
    assert G <= 512
    xgt = nc.dram_tensor("xgt", [128, G * 128], F16, kind="ExternalInput").ap()
    xpm = nc.dram_tensor("xpm", [128, NLOC], F16, kind="ExternalInput").ap()
    wlt = nc.dram_tensor("wlt", [128, C], F16, kind="ExternalInput").ap()
    wrt = nc.dram_tensor("wrt", [128, C], F16, kind="ExternalInput").ap()
    attv = nc.dram_tensor("attv", [C, 1], F16, kind="ExternalInput").ap()
    ebias = nc.dram_tensor("ebias", [C, 1], F32, kind="ExternalInput").ap()
    obias = nc.dram_tensor("obias", [C, 1], F32, kind="ExternalInput").ap()
    mask_in = nc.dram_tensor("mask", [128, G], F16, kind="ExternalInput").ap()
    ident_in = nc.dram_tensor("ident", [128, 128], F16, kind="ExternalInput").ap()
    hout = nc.dram_tensor("hout", [C, NLOC], F32, kind="ExternalOutput").ap()

    with tile.TileContext(nc) as tc, ExitStack() as ctx:
        const = ctx.enter_context(tc.tile_pool(name="c", bufs=1))
        xgp = ctx.enter_context(tc.tile_pool(name="xg", bufs=2))
        xlp = ctx.enter_context(tc.tile_pool(name="xl", bufs=2))
        shp = ctx.enter_context(tc.tile_pool(name="sh", bufs=2))
        wp = ctx.enter_context(tc.tile_pool(name="w", bufs=2))
        ap_ = ctx.enter_context(tc.tile_pool(name="al", bufs=2))
        psx = ctx.enter_context(tc.tile_pool(name="psx", bufs=2, space="PSUM"))
        psa = ctx.enter_context(tc.tile_pool(name="psa", bufs=2, space="PSUM"))
        outp = ctx.enter_context(tc.tile_pool(name="o", bufs=1))

        t_wlt = const.tile([128, C], F16)
        nc.sync.dma_start(t_wlt[:], wlt[:])
        t_wrt = const.tile([128, C], F16)
        nc.sync.dma_start(t_wrt[:], wrt[:])
        t_att = const.tile([C, 1], F16)
        nc.sync.dma_start(t_att[:], attv[:])
        t_eb = const.tile([C, 1], F32)
        nc.sync.dma_start(t_eb[:], ebias[:])
        t_ob = const.tile([C, 1], F32)
        nc.sync.dma_start(t_ob[:], obias[:])
        t_mask = const.tile([128, G], F16)
        nc.sync.dma_start(t_mask[:], mask_in[:])
        t_id = const.tile([128, 128], F16)
        nc.sync.dma_start(t_id[:], ident_in[:])
        t_xpm = const.tile([128, NLOC], F16)
        nc.sync.dma_start(t_xpm[:], xpm[:])

        t_out = outp.tile([C, NLOC], F32)
        SC = 4  # psum chunk: 4 j-tiles = 512 cols
        DMAX = int(Dc.max())

        for k in range(ROWS):
            Dk, Wk = int(Dc[k]), int(W[k])
            ncols = Dk * 128
            t_xg = xgp.tile([128, DMAX * 128], F16, tag="xg")
            nc.sync.dma_start(t_xg[:, :ncols], xgt[:, Wk*128:(Wk+Dk)*128])
            t_xls = xlp.tile([C, DMAX * 128], F16, tag="xls")
            t_ep = psx.tile([128, DMAX], F32, tag="pe")
            for c0 in range(0, Dk, SC):
                w = min(SC, Dk - c0)
                cw = w * 128
                # S(psum) = Wl @ xg + Wr @ xpm_bcast
                t_px = psx.tile([C, 512], F32, tag="px")
                nc.tensor.matmul(t_px[:, :cw], t_wlt[:],
                                 t_xg[:, (c0*128):(c0*128)+cw],
                                 start=True, stop=False)
                xb = t_xpm[:, k*128:(k+1)*128].unsqueeze(1) \
                    .broadcast_to([128, w, 128])
                nc.tensor.matmul(t_px[:, :cw].rearrange("a (b c) -> a b c", c=128),
                                 t_wrt[:], xb, start=False, stop=True)
                # H = lrelu(S + ebias) -> f16 (ACT Lrelu's alpha is broken on
                # HW -> DVE: se = S+eb ; h0 = se*0.2 ; H = max(se, h0))
                t_se = shp.tile([C, 512], F16, tag="se")
                nc.vector.tensor_scalar_add(t_se[:, :cw], t_px[:, :cw], t_eb[:])
                t_h0 = shp.tile([C, 512], F16, tag="h0")
                nc.vector.tensor_scalar_mul(t_h0[:, :cw], t_se[:, :cw], NEG_SLOPE)
                t_h = shp.tile([C, 512], F16, tag="h")
                nc.vector.tensor_tensor(t_h[:, :cw], t_se[:, :cw], t_h0[:, :cw],
                                        op=mybir.AluOpType.max)
                # XLs(f16) via Wl-only matmul
                t_px2 = psx.tile([C, 512], F32, tag="px2")
                nc.tensor.matmul(t_px2[:, :cw], t_wlt[:],
                                 t_xg[:, (c0*128):(c0*128)+cw],
                                 start=True, stop=True)
                nc.scalar.activation(t_xls[:, (c0*128):(c0*128)+cw],
                                     t_px2[:, :cw],
                                     mybir.ActivationFunctionType.Copy)
                # e per 128-col tile: lhsT = H-tile, rhs = att -> psum col
                for t in range(w):
                    nc.tensor.matmul(t_ep[:, c0+t:c0+t+1],
                                     t_h[:, t*128:(t+1)*128], t_att[:],
                                     start=True, stop=True)
            # per-row softmax: a = exp(e)*mask ; alpha = a/(sum+eps)
            t_e = ap_.tile([128, DMAX], F32, tag="ee")
            nc.scalar.activation(t_e[:, :Dk], t_ep[:, :Dk],
                                 mybir.ActivationFunctionType.Exp)
            t_a = ap_.tile([128, DMAX], F32, tag="aa")
            nc.vector.tensor_mul(t_a[:, :Dk], t_e[:, :Dk], t_mask[:, Wk:Wk+Dk])
            t_s = ap_.tile([128, 1], F32, tag="ssum")
            nc.vector.tensor_reduce(t_s[:], t_a[:, :Dk],
                                    axis=mybir.AxisListType.X,
                                    op=mybir.AluOpType.add)
            t_sr = ap_.tile([128, 1], F32, tag="sr")
            nc.vector.tensor_scalar_add(t_sr[:], t_s[:], 1e-16)
            t_r = ap_.tile([128, 1], F32, tag="rr")
            nc.vector.reciprocal(t_r[:], t_sr[:])
            t_al16 = ap_.tile([128, DMAX], F16, tag="al16")
            nc.vector.tensor_scalar_mul(t_al16[:, :Dk], t_a[:, :Dk], t_r[:])
            # aggregation: aRep per-tile + W-mult (d-innermost) + reduce
            t_w = wp.tile([C, 128, DMAX], F16, tag="wv")
            for c0 in range(0, Dk, SC):
                w = min(SC, Dk - c0)
                cw = w * 128
                t_pa = psa.tile([128, 512], F32, tag="pa")
                for t in range(w):
                    nc.tensor.matmul(
                        t_pa[:, t*128:(t+1)*128],
                        t_al16[:, c0+t:c0+t+1].broadcast_to([128, 128]),
                        t_id[:], start=True, stop=True)
                wout = t_w[:, :, c0:c0+w].transpose([0, 2, 1])
                nc.vector.tensor_mul(
                    wout,
                    t_xls[:, c0*128:(c0*128)+cw].rearrange(
                        "a (b c) -> a b c", c=128),
                    t_pa[:C, :cw].rearrange("a (b c) -> a b c", c=128))
            nc.vector.tensor_reduce(t_out[:, k*128:(k+1)*128],
                                    t_w[:, :, :Dk],
                                    axis=mybir.AxisListType.X,
                                    op=mybir.AluOpType.add)
        nc.vector.tensor_scalar_add(t_out[:], t_out[:], t_ob[:])
        nc.sync.dma_start(hout[:], t_out[:])
    nc.compile()
    return nc


def _build_adj_neff():
    nc = bacc.Bacc("TRN2", target_bir_lowering=False, debug=False,
                   num_devices=NC)
    zT = nc.dram_tensor("zT", [OUT, N], BF16, kind="ExternalInput").ap()
    zTl = nc.dram_tensor("zTl", [OUT, NLOC], BF16, kind="ExternalInput").ap()
    adj = nc.dram_tensor("adj", [NLOC, N], F16, kind="ExternalOutput").ap()
    NCH = 512
    with tile.TileContext(nc) as tc, ExitStack() as ctx:
        const = ctx.enter_context(tc.tile_pool(name="c", bufs=1))
        obuf = ctx.enter_context(tc.tile_pool(name="o", bufs=3))
        psum = ctx.enter_context(tc.tile_pool(name="ps", bufs=8, space="PSUM"))
        t_z = const.tile([OUT, N], BF16)
        nc.sync.dma_start(t_z[:], zT[:])
        t_zl = const.tile([OUT, NLOC], BF16)
        nc.sync.dma_start(t_zl[:], zTl[:])
        for mt in range(NLOC // 128):
            t_o = obuf.tile([128, N], F16, tag="o")
            for cc in range(N // NCH):
                t_ps = psum.tile([128, NCH], F32, tag="ps")
                nc.tensor.matmul(t_ps[:], t_zl[:, bass.ts(mt, 128)],
                                 t_z[:, bass.ts(cc, NCH)], start=True, stop=True)
                nc.scalar.activation(t_o[:, bass.ts(cc, NCH)], t_ps[:],
                                     mybir.ActivationFunctionType.Sigmoid)
            nc.sync.dma_start(adj[bass.ts(mt, 128), :], t_o[:])
    nc.compile()
    return nc


def _gat_layer(cores, Dc, W, G, x_feat, Wl, bl, Wr, br, att, b, relu):
    """One GAT layer. x_feat [N, 128-padded] f32. Returns out [N, C] f32."""
    C = Wl.shape[0]
    xl = x_feat @ Wl.T
    xr = x_feat @ Wr.T
    if HOST_GAT:
        h = _gat_host(cores, G, W, Dc, xl.astype(np.float16).astype(np.float32),
                      xr + (br + bl), att, bl + b, relu)
        return h
    key = ("gat", C, tuple(Dc))
    if key not in _cache:
        _cache[key] = _build_gat_neff(Dc, C)
    ncg = _cache[key]
    xT16 = np.ascontiguousarray(x_feat.T).astype(np.float16)  # [128, N]
    in_maps = []
    for lay in cores:
        in_maps.append({
            "xgt": np.ascontiguousarray(xT16[:, lay["gsrc"]]),
            "xpm": np.ascontiguousarray(xT16[:, lay["node_of"]]),
            "wlt": np.ascontiguousarray(Wl.T).astype(np.float16),
            "wrt": np.ascontiguousarray(Wr.T).astype(np.float16),
            "attv": att.reshape(C, 1).astype(np.float16),
            "ebias": (bl + br).reshape(C, 1).astype(np.float32),
            "obias": (bl + b).reshape(C, 1).astype(np.float32),
            "mask": lay["mask"],
            "ident": np.eye(128, dtype=np.float16),
        })
    res = run_bass_kernel_spmd(ncg, in_maps, list(range(NC)))
    global LAST_HW_NS
    LAST_HW_NS += _sim_ns(key, ncg)
    out = np.zeros((N, C), dtype=np.float32)
    for ci, lay in enumerate(cores):
        hT = res.results[ci]["hout"]          # [C, NLOC]
        out[lay["node_of"], :] = hT.T
    if relu:
        out = np.maximum(out, 0.0)
    return out


def kernel(x, edge_index, Wl1, bl1, Wr1, br1, att1, b1,
           Wl2, bl2, Wr2, br2, att2, b2):
    x = np.asarray(x, dtype=np.float32)
    edge_index = np.asarray(edge_index)
    Wl1, bl1, Wr1, br1, att1, b1 = [np.asarray(a, np.float32) for a in
                                    (Wl1, bl1, Wr1, br1, att1, b1)]
    Wl2, bl2, Wr2, br2, att2, b2 = [np.asarray(a, np.float32) for a in
                                    (Wl2, bl2, Wr2, br2, att2, b2)]
    global LAST_HW_NS
    LAST_HW_NS = 0
    ek = hash(edge_index.tobytes())
    if ("layout", ek) not in _cache:
        _cache[("layout", ek)] = _edge_layout(edge_index)
    cores, Dc, W, G = _cache[("layout", ek)]

    h = _gat_layer(cores, Dc, W, G, x, Wl1, bl1, Wr1, br1, att1, b1, True)
    h_pad = np.zeros((N, 128), dtype=np.float32)
    h_pad[:, :HID] = h
    out2 = _gat_layer(cores, Dc, W, G, h_pad, Wl2, bl2, Wr2, br2, att2, b2,
                      False)

    import jax, jax.numpy as jnp
    cpu = jax.devices("cpu")[0]
    with jax.default_device(cpu):
        alpha = np.asarray(jax.nn.softplus(
            jax.device_put(jnp.asarray(out2), cpu))) + 1e-6
        g = jax.random.gamma(jax.random.key(42), jnp.asarray(alpha))
        z = np.asarray(g / jnp.sum(g, axis=-1, keepdims=True),
                       dtype=np.float32)
    alpha = alpha.astype(np.float32)

    if "adj" not in _cache:
        _cache["adj"] = _build_adj_neff()
    nca = _cache["adj"]
    import ml_dtypes
    zT16 = np.ascontiguousarray(z.T).astype(ml_dtypes.bfloat16)
    in_maps = [{"zT": zT16,
                "zTl": np.ascontiguousarray(zT16[:, ci*NLOC:(ci+1)*NLOC])}
               for ci in range(NC)]
    res = run_bass_kernel_spmd(nca, in_maps, list(range(NC)))
    LAST_HW_NS += _sim_ns("adj", nca)
    adj = np.empty((N, N), dtype=np.float32)
    for ci in range(NC):
        adj[ci*NLOC:(ci+1)*NLOC, :] = res.results[ci]["adj"]
    return adj, alpha, z


# revision 20
# speedup vs baseline: 1.1436x; 1.1436x over previous
"""DirVGAE (GATv2 x2 -> Dirichlet z -> sigmoid(z z^T)) on 8 trn2 NeuronCores.

Sharding: dst-nodes row-sharded across 8 cores (1536 rows each); GAT weights
replicated; edge lists partitioned by destination core; dense sigmoid(z z^T)
row-sharded the same way.

GAT layer on device (transposed layout, host-staged gathers):
  The slot->src map is data-independent, so the source-feature "halo
  exchange" is staged on host: ship x^T[:, src(slot)] (f16) per core; the
  device applies Wl via one stationary matmul (gather commutes with the
  linear transform), adds the xr broadcast, does LeakyReLU + attention dot
  (per-tile PE matmuls), segment softmax over per-partition windows, and
  aggregates via alpha-replicate (per-tile transpose-LDW) + windowed reduce.
Adjacency: bf16 z @ z^T on PE + sigmoid evac (f16 out).
Gamma sampling: jax CPU (bit-matches the reference sampler).
"""
import sys, os, time
sys.path.insert(0, "/opt/trn_rl_repo")
import numpy as np
from contextlib import ExitStack

import concourse.bass as bass
import concourse.tile as tile
from concourse import bacc, mybir
from concourse.bass_utils import run_bass_kernel_spmd

F32 = mybir.dt.float32
F16 = mybir.dt.float16
BF16 = mybir.dt.bfloat16

N, E = 12288, 393216
IN, HID, OUT = 128, 128, 64
NEG_SLOPE = 0.2
NC = 8
NLOC = N // NC
ROWS = NLOC // 128

HOST_GAT = os.environ.get("HOST_GAT", "0") == "1"

_cache = {}
LAST_HW_NS = 0
_neff_ns = {}


def _sim_ns(key, nc):
    """Cost-model estimate of one NEFF execution (per core), cached."""
    if key not in _neff_ns:
        try:
            from concourse.timeline_sim import TimelineSim
            _neff_ns[key] = int(TimelineSim(nc).simulate())
        except Exception:
            _neff_ns[key] = 0
    return _neff_ns[key]


def _edge_layout(edge_index):
    """Per-core slot layout: nodes ranked by padded degree, rows of 128,
    common D per row; slot s = j*128 + p holds edge d=(j-W_k) of node
    rank k*128+p. Returns per-core dicts + common D_row."""
    src = edge_index[0].astype(np.int64)
    dst = edge_index[1].astype(np.int64)
    cores = []
    for ci in range(NC):
        r0 = ci * NLOC
        sel = (dst >= r0) & (dst < r0 + NLOC)
        s_i = src[sel]
        d_i = dst[sel] - r0
        order = np.argsort(d_i, kind="stable")
        s_i = s_i[order]
        d_i = d_i[order]
        deg = np.bincount(d_i, minlength=NLOC)
        starts = np.concatenate([[0], np.cumsum(deg)])
        Dpad = np.maximum(4, ((deg + 3) // 4) * 4)
        rank_of = np.argsort(Dpad, kind="stable")
        D_row = np.array([Dpad[rank_of[k*128:(k+1)*128]].max() for k in range(ROWS)])
        cores.append(dict(r0=r0, s_i=s_i, deg=deg, starts=starts,
                          rank_of=rank_of, D_row=D_row))
    Dc = np.stack([c["D_row"] for c in cores]).max(axis=0)
    W = np.concatenate([[0], np.cumsum(Dc)])
    G = int(W[-1])
    for lay in cores:
        deg, starts, rank_of, s_i = (lay["deg"], lay["starts"],
                                     lay["rank_of"], lay["s_i"])
        gsrc = np.zeros(G * 128, dtype=np.int32)
        mask = np.zeros((128, G), dtype=np.float16)
        node_of = np.zeros(NLOC, dtype=np.int64)
        for k in range(ROWS):
            Dk = int(Dc[k])
            Wk = int(W[k])
            nodes = rank_of[k * 128:(k + 1) * 128]
            node_of[k * 128:(k + 1) * 128] = nodes + lay["r0"]
            for p in range(128):
                n = nodes[p]
                d = int(deg[n])
                e0 = int(starts[n])
                sl = s_i[e0:e0 + d]
                jj = (Wk + np.arange(d)) * 128 + p
                gsrc[jj] = sl
                mask[p, Wk:Wk + d] = 1.0
        lay["gsrc"] = gsrc
        lay["mask"] = mask
        lay["node_of"] = node_of
    return cores, Dc, W, G


def _gat_host(cores, G, W, Dc, xl, xr, att, bias_out, relu):
    """Host fallback of the device layer (same math, f32)."""
    C = xl.shape[1]
    out = np.zeros((N, C), dtype=np.float32)
    for lay in cores:
        xg = xl[lay["gsrc"], :]                       # [G*128, C]
        xrv = xr[lay["node_of"], :]                   # [1536, C]
        mask = lay["mask"].astype(np.float32)
        e = np.zeros((128, G), dtype=np.float32)
        for k in range(ROWS):
            Dk, Wk = int(Dc[k]), int(W[k])
            blk = xg[Wk*128:(Wk+Dk)*128].reshape(Dk, 128, C)
            s = blk + xrv[k*128:(k+1)*128][None, :, :]
            h = np.where(s > 0, s, NEG_SLOPE * s)
            e[:, Wk:Wk+Dk] = (h @ att).T
        a = np.exp(e) * mask
        for k in range(ROWS):
            Dk, Wk = int(Dc[k]), int(W[k])
            aw = a[:, Wk:Wk+Dk]
            al = aw / (aw.sum(1, keepdims=True) + 1e-16)
            blk = xg[Wk*128:(Wk+Dk)*128].reshape(Dk, 128, C)
            o = (al.T[:, :, None] * blk).sum(0) + bias_out
            out[lay["node_of"][k*128:(k+1)*128]] = np.maximum(o, 0) if relu else o
    return out


def _build_gat_neff(Dc, C):
    nc = bacc.Bacc("TRN2", target_bir_lowering=False, debug=False,
                   num_devices=NC)
    W = np.concatenate([[0], np.cumsum(Dc)])
    G = int(W[-1])
    assert G <= 512
    xgt = nc.dram_tensor("xgt", [128, G * 128], F16, kind="ExternalInput").ap()
    xpm = nc.dram_tensor("xpm", [128, NLOC], F16, kind="ExternalInput").ap()
    wlt = nc.dram_tensor("wlt", [128, C], F16, kind="ExternalInput").ap()
    wrt = nc.dram_tensor("wrt", [128, C], F16, kind="ExternalInput").ap()
    attv = nc.dram_tensor("attv", [C, 1], F16, kind="ExternalInput").ap()
    ebias = nc.dram_tensor("ebias", [C, 1], F32, kind="ExternalInput").ap()
    obias = nc.dram_tensor("obias", [C, 1], F32, kind="ExternalInput").ap()
    mask_in = nc.dram_tensor("mask", [128, G], F16, kind="ExternalInput").ap()
    ident_in = nc.dram_tensor("ident", [128, 128], F16, kind="ExternalInput").ap()
    hout = nc.dram_tensor("hout", [C, NLOC], F32, kind="ExternalOutput").ap()

    with tile.TileContext(nc) as tc, ExitStack() as ctx:
        const = ctx.enter_context(tc.tile_pool(name="c", bufs=1))
        xgp = ctx.enter_context(tc.tile_pool(name="xg", bufs=2))
        xlp = ctx.enter_context(tc.tile_pool(name="xl", bufs=2))
        shp = ctx.enter_context(tc.tile_pool(name="sh", bufs=2))
        wp = ctx.enter_context(tc.tile_pool(name="w", bufs=2))
        ap_ = ctx.enter_context(tc.tile_pool(name="al", bufs=2))
        psx = ctx.enter_context(tc.tile_pool(name="psx", bufs=2, space="PSUM"))
        psa = ctx.enter_context(tc.tile_pool(name="psa", bufs=2, space="PSUM"))
        outp = ctx.enter_context(tc.tile_pool(name="o", bufs=1))

        t_wlt = const.tile([128, C], F16)
        nc.sync.dma_start(t_wlt[:], wlt[:])
        t_wrt = const.tile([128, C], F16)
        nc.sync.dma_start(t_wrt[:], wrt[:])
        t_att = const.tile([C, 1], F16)
        nc.sync.dma_start(t_att[:], attv[:])
        t_eb = const.tile([C, 1], F32)
        nc.sync.dma_start(t_eb[:], ebias[:])
        t_ob = const.tile([C, 1], F32)
        nc.sync.dma_start(t_ob[:], obias[:])
        t_mask = const.tile([128, G], F16)
        nc.sync.dma_start(t_mask[:], mask_in[:])
        t_id = const.tile([128, 128], F16)
        nc.sync.dma_start(t_id[:], ident_in[:])
        t_xpm = const.tile([128, NLOC], F16)
        nc.sync.dma_start(t_xpm[:], xpm[:])

        t_out = outp.tile([C, NLOC], F32)
        SC = 4  # psum chunk: 4 j-tiles = 512 cols
        DMAX = int(Dc.max())

        # xr_T = Wr @ xpm (+ ebias), computed once: [C, NLOC] f16
        t_xr = const.tile([C, NLOC], F16)
        for i in range(NLOC // 512):
            t_pxr = psx.tile([C, 512], F32, tag="px")
            nc.tensor.matmul(t_pxr[:], t_wrt[:], t_xpm[:, bass.ts(i, 512)],
                             start=True, stop=True)
            nc.vector.tensor_scalar_add(t_xr[:, bass.ts(i, 512)], t_pxr[:],
                                        t_eb[:])

        for k in range(ROWS):
            Dk, Wk = int(Dc[k]), int(W[k])
            ncols = Dk * 128
            t_xg = xgp.tile([128, DMAX * 128], F16, tag="xg")
            nc.sync.dma_start(t_xg[:, :ncols], xgt[:, Wk*128:(Wk+Dk)*128])
            t_xls = xlp.tile([C, DMAX * 128], F16, tag="xls")
            t_ep = psx.tile([128, DMAX], F32, tag="pe")
            for c0 in range(0, Dk, SC):
                w = min(SC, Dk - c0)
                cw = w * 128
                # XLs(psum) = Wl @ xg ; evac f16
                t_px = psx.tile([C, 512], F32, tag="px")
                nc.tensor.matmul(t_px[:, :cw], t_wlt[:],
                                 t_xg[:, (c0*128):(c0*128)+cw],
                                 start=True, stop=True)
                nc.scalar.activation(t_xls[:, (c0*128):(c0*128)+cw],
                                     t_px[:, :cw],
                                     mybir.ActivationFunctionType.Copy)
                # S = XLs + xr_bcast ; H = max(S, 0.2*S)  (all f16 on DVE)
                t_se = shp.tile([C, 512], F16, tag="se")
                xrb = t_xr[:, k*128:(k+1)*128].unsqueeze(1) \
                    .broadcast_to([C, w, 128])
                nc.vector.tensor_add(
                    t_se[:, :cw].rearrange("a (b c) -> a b c", c=128),
                    t_xls[:, (c0*128):(c0*128)+cw].rearrange(
                        "a (b c) -> a b c", c=128), xrb)
                t_h0 = shp.tile([C, 512], F16, tag="h0")
                nc.vector.tensor_scalar_mul(t_h0[:, :cw], t_se[:, :cw], NEG_SLOPE)
                t_h = shp.tile([C, 512], F16, tag="h")
                nc.vector.tensor_tensor(t_h[:, :cw], t_se[:, :cw], t_h0[:, :cw],
                                        op=mybir.AluOpType.max)
                # e per 128-col tile: lhsT = H-tile, rhs = att -> psum col
                for t in range(w):
                    nc.tensor.matmul(t_ep[:, c0+t:c0+t+1],
                                     t_h[:, t*128:(t+1)*128], t_att[:],
                                     start=True, stop=True)
            # per-row softmax: a = exp(e)*mask ; alpha = a/(sum+eps)
            t_e = ap_.tile([128, DMAX], F32, tag="ee")
            nc.scalar.activation(t_e[:, :Dk], t_ep[:, :Dk],
                                 mybir.ActivationFunctionType.Exp)
            t_a = ap_.tile([128, DMAX], F32, tag="aa")
            nc.vector.tensor_mul(t_a[:, :Dk], t_e[:, :Dk], t_mask[:, Wk:Wk+Dk])
            t_s = ap_.tile([128, 1], F32, tag="ssum")
            nc.vector.tensor_reduce(t_s[:], t_a[:, :Dk],
                                    axis=mybir.AxisListType.X,
                                    op=mybir.AluOpType.add)
            t_sr = ap_.tile([128, 1], F32, tag="sr")
            nc.vector.tensor_scalar_add(t_sr[:], t_s[:], 1e-16)
            t_r = ap_.tile([128, 1], F32, tag="rr")
            nc.vector.reciprocal(t_r[:], t_sr[:])
            t_al16 = ap_.tile([128, DMAX], F16, tag="al16")
            nc.vector.tensor_scalar_mul(t_al16[:, :Dk], t_a[:, :Dk], t_r[:])
            # aggregation: aRep per-tile + W-mult (d-innermost) + reduce
            t_w = wp.tile([C, 128, DMAX], F16, tag="wv")
            for c0 in range(0, Dk, SC):
                w = min(SC, Dk - c0)
                cw = w * 128
                t_pa = psa.tile([128, 512], F32, tag="pa")
                for t in range(w):
                    nc.tensor.matmul(
                        t_pa[:, t*128:(t+1)*128],
                        t_al16[:, c0+t:c0+t+1].broadcast_to([128, 128]),
                        t_id[:], start=True, stop=True)
                wout = t_w[:, :, c0:c0+w].transpose([0, 2, 1])
                nc.vector.tensor_mul(
                    wout,
                    t_xls[:, c0*128:(c0*128)+cw].rearrange(
                        "a (b c) -> a b c", c=128),
                    t_pa[:C, :cw].rearrange("a (b c) -> a b c", c=128))
            nc.vector.tensor_reduce(t_out[:, k*128:(k+1)*128],
                                    t_w[:, :, :Dk],
                                    axis=mybir.AxisListType.X,
                                    op=mybir.AluOpType.add)
        nc.vector.tensor_scalar_add(t_out[:], t_out[:], t_ob[:])
        nc.sync.dma_start(hout[:], t_out[:])
    nc.compile()
    return nc


def _build_adj_neff():
    nc = bacc.Bacc("TRN2", target_bir_lowering=False, debug=False,
                   num_devices=NC)
    zT = nc.dram_tensor("zT", [OUT, N], BF16, kind="ExternalInput").ap()
    zTl = nc.dram_tensor("zTl", [OUT, NLOC], BF16, kind="ExternalInput").ap()
    adj = nc.dram_tensor("adj", [NLOC, N], F16, kind="ExternalOutput").ap()
    NCH = 512
    with tile.TileContext(nc) as tc, ExitStack() as ctx:
        const = ctx.enter_context(tc.tile_pool(name="c", bufs=1))
        obuf = ctx.enter_context(tc.tile_pool(name="o", bufs=3))
        psum = ctx.enter_context(tc.tile_pool(name="ps", bufs=4, space="PSUM"))
        t_z = const.tile([OUT, N], BF16)
        nc.sync.dma_start(t_z[:], zT[:])
        t_zl = const.tile([OUT, NLOC], BF16)
        nc.sync.dma_start(t_zl[:], zTl[:])
        for mt in range(NLOC // 128):
            t_o = obuf.tile([128, N], F16, tag="o")
            for cc in range(N // (2 * NCH)):
                t_ps = psum.tile([128, 2 * NCH], F32, tag="ps")
                nc.tensor.matmul(t_ps[:, :NCH], t_zl[:, bass.ts(mt, 128)],
                                 t_z[:, bass.ts(2 * cc, NCH)],
                                 start=True, stop=True)
                nc.tensor.matmul(t_ps[:, NCH:], t_zl[:, bass.ts(mt, 128)],
                                 t_z[:, bass.ts(2 * cc + 1, NCH)],
                                 start=True, stop=True)
                nc.scalar.activation(t_o[:, bass.ts(cc, 2 * NCH)], t_ps[:],
                                     mybir.ActivationFunctionType.Sigmoid)
            nc.sync.dma_start(adj[bass.ts(mt, 128), :], t_o[:])
    nc.compile()
    return nc


def _gat_layer(cores, Dc, W, G, x_feat, Wl, bl, Wr, br, att, b, relu):
    """One GAT layer. x_feat [N, 128-padded] f32. Returns out [N, C] f32."""
    C = Wl.shape[0]
    xl = x_feat @ Wl.T
    xr = x_feat @ Wr.T
    if HOST_GAT:
        h = _gat_host(cores, G, W, Dc, xl.astype(np.float16).astype(np.float32),
                      xr + (br + bl), att, bl + b, relu)
        return h
    key = ("gat", C, tuple(Dc))
    if key not in _cache:
        _cache[key] = _build_gat_neff(Dc, C)
    ncg = _cache[key]
    xT16 = np.ascontiguousarray(x_feat.T).astype(np.float16)  # [128, N]
    in_maps = []
    for lay in cores:
        in_maps.append({
            "xgt": np.ascontiguousarray(xT16[:, lay["gsrc"]]),
            "xpm": np.ascontiguousarray(xT16[:, lay["node_of"]]),
            "wlt": np.ascontiguousarray(Wl.T).astype(np.float16),
            "wrt": np.ascontiguousarray(Wr.T).astype(np.float16),
            "attv": att.reshape(C, 1).astype(np.float16),
            "ebias": (bl + br).reshape(C, 1).astype(np.float32),
            "obias": (bl + b).reshape(C, 1).astype(np.float32),
            "mask": lay["mask"],
            "ident": np.eye(128, dtype=np.float16),
        })
    res = run_bass_kernel_spmd(ncg, in_maps, list(range(NC)))
    global LAST_HW_NS
    LAST_HW_NS += _sim_ns(key, ncg)
    out = np.zeros((N, C), dtype=np.float32)
    for ci, lay in enumerate(cores):
        hT = res.results[ci]["hout"]          # [C, NLOC]
        out[lay["node_of"], :] = hT.T
    if relu:
        out = np.maximum(out, 0.0)
    return out


def kernel(x, edge_index, Wl1, bl1, Wr1, br1, att1, b1,
           Wl2, bl2, Wr2, br2, att2, b2):
    x = np.asarray(x, dtype=np.float32)
    edge_index = np.asarray(edge_index)
    Wl1, bl1, Wr1, br1, att1, b1 = [np.asarray(a, np.float32) for a in
                                    (Wl1, bl1, Wr1, br1, att1, b1)]
    Wl2, bl2, Wr2, br2, att2, b2 = [np.asarray(a, np.float32) for a in
                                    (Wl2, bl2, Wr2, br2, att2, b2)]
    global LAST_HW_NS
    LAST_HW_NS = 0
    ek = hash(edge_index.tobytes())
    if ("layout", ek) not in _cache:
        _cache[("layout", ek)] = _edge_layout(edge_index)
    cores, Dc, W, G = _cache[("layout", ek)]

    h = _gat_layer(cores, Dc, W, G, x, Wl1, bl1, Wr1, br1, att1, b1, True)
    h_pad = np.zeros((N, 128), dtype=np.float32)
    h_pad[:, :HID] = h
    out2 = _gat_layer(cores, Dc, W, G, h_pad, Wl2, bl2, Wr2, br2, att2, b2,
                      False)

    import jax, jax.numpy as jnp
    cpu = jax.devices("cpu")[0]
    with jax.default_device(cpu):
        alpha = np.asarray(jax.nn.softplus(
            jax.device_put(jnp.asarray(out2), cpu))) + 1e-6
        g = jax.random.gamma(jax.random.key(42), jnp.asarray(alpha))
        z = np.asarray(g / jnp.sum(g, axis=-1, keepdims=True),
                       dtype=np.float32)
    alpha = alpha.astype(np.float32)

    if "adj" not in _cache:
        _cache["adj"] = _build_adj_neff()
    nca = _cache["adj"]
    import ml_dtypes
    zT16 = np.ascontiguousarray(z.T).astype(ml_dtypes.bfloat16)
    in_maps = [{"zT": zT16,
                "zTl": np.ascontiguousarray(zT16[:, ci*NLOC:(ci+1)*NLOC])}
               for ci in range(NC)]
    res = run_bass_kernel_spmd(nca, in_maps, list(range(NC)))
    LAST_HW_NS += _sim_ns("adj", nca)
    adj = np.empty((N, N), dtype=np.float32)
    for ci in range(NC):
        adj[ci*NLOC:(ci+1)*NLOC, :] = res.results[ci]["adj"]
    return adj, alpha, z


# revision 21
# speedup vs baseline: 1.1438x; 1.0002x over previous
"""DirVGAE (GATv2 x2 -> Dirichlet z -> sigmoid(z z^T)) on 8 trn2 NeuronCores.

Sharding: dst-nodes row-sharded across 8 cores (1536 rows each); GAT weights
replicated; edge lists partitioned by destination core; dense sigmoid(z z^T)
row-sharded the same way.

GAT layer on device (transposed layout, host-staged gathers):
  The slot->src map is data-independent, so the source-feature "halo
  exchange" is staged on host: ship x^T[:, src(slot)] (f16) per core; the
  device applies Wl via one stationary matmul (gather commutes with the
  linear transform), adds the xr broadcast, does LeakyReLU + attention dot
  (per-tile PE matmuls), segment softmax over per-partition windows, and
  aggregates via alpha-replicate (per-tile transpose-LDW) + windowed reduce.
Adjacency: bf16 z @ z^T on PE + sigmoid evac (f16 out).
Gamma sampling: jax CPU (bit-matches the reference sampler).
"""
import sys, os, time
sys.path.insert(0, "/opt/trn_rl_repo")
import numpy as np
from contextlib import ExitStack

import concourse.bass as bass
import concourse.tile as tile
from concourse import bacc, mybir
from concourse.bass_utils import run_bass_kernel_spmd

F32 = mybir.dt.float32
F16 = mybir.dt.float16
BF16 = mybir.dt.bfloat16

N, E = 12288, 393216
IN, HID, OUT = 128, 128, 64
NEG_SLOPE = 0.2
NC = 8
NLOC = N // NC
ROWS = NLOC // 128

HOST_GAT = os.environ.get("HOST_GAT", "0") == "1"

_cache = {}
LAST_HW_NS = 0
_neff_ns = {}


def _sim_ns(key, nc):
    """Cost-model estimate of one NEFF execution (per core), cached."""
    if key not in _neff_ns:
        try:
            from concourse.timeline_sim import TimelineSim
            _neff_ns[key] = int(TimelineSim(nc).simulate())
        except Exception:
            _neff_ns[key] = 0
    return _neff_ns[key]


def _edge_layout(edge_index):
    """Per-core slot layout: nodes ranked by padded degree, rows of 128,
    common D per row; slot s = j*128 + p holds edge d=(j-W_k) of node
    rank k*128+p. Returns per-core dicts + common D_row."""
    src = edge_index[0].astype(np.int64)
    dst = edge_index[1].astype(np.int64)
    cores = []
    for ci in range(NC):
        r0 = ci * NLOC
        sel = (dst >= r0) & (dst < r0 + NLOC)
        s_i = src[sel]
        d_i = dst[sel] - r0
        order = np.argsort(d_i, kind="stable")
        s_i = s_i[order]
        d_i = d_i[order]
        deg = np.bincount(d_i, minlength=NLOC)
        starts = np.concatenate([[0], np.cumsum(deg)])
        Dpad = np.maximum(4, ((deg + 3) // 4) * 4)
        rank_of = np.argsort(Dpad, kind="stable")
        D_row = np.array([Dpad[rank_of[k*128:(k+1)*128]].max() for k in range(ROWS)])
        cores.append(dict(r0=r0, s_i=s_i, deg=deg, starts=starts,
                          rank_of=rank_of, D_row=D_row))
    Dc = np.stack([c["D_row"] for c in cores]).max(axis=0)
    W = np.concatenate([[0], np.cumsum(Dc)])
    G = int(W[-1])
    for lay in cores:
        deg, starts, rank_of, s_i = (lay["deg"], lay["starts"],
                                     lay["rank_of"], lay["s_i"])
        gsrc = np.zeros(G * 128, dtype=np.int32)
        mask = np.zeros((128, G), dtype=np.float16)
        node_of = np.zeros(NLOC, dtype=np.int64)
        for k in range(ROWS):
            Dk = int(Dc[k])
            Wk = int(W[k])
            nodes = rank_of[k * 128:(k + 1) * 128]
            node_of[k * 128:(k + 1) * 128] = nodes + lay["r0"]
            for p in range(128):
                n = nodes[p]
                d = int(deg[n])
                e0 = int(starts[n])
                sl = s_i[e0:e0 + d]
                jj = (Wk + np.arange(d)) * 128 + p
                gsrc[jj] = sl
                mask[p, Wk:Wk + d] = 1.0
        lay["gsrc"] = gsrc
        lay["mask"] = mask
        lay["node_of"] = node_of
    return cores, Dc, W, G


def _gat_host(cores, G, W, Dc, xl, xr, att, bias_out, relu):
    """Host fallback of the device layer (same math, f32)."""
    C = xl.shape[1]
    out = np.zeros((N, C), dtype=np.float32)
    for lay in cores:
        xg = xl[lay["gsrc"], :]                       # [G*128, C]
        xrv = xr[lay["node_of"], :]                   # [1536, C]
        mask = lay["mask"].astype(np.float32)
        e = np.zeros((128, G), dtype=np.float32)
        for k in range(ROWS):
            Dk, Wk = int(Dc[k]), int(W[k])
            blk = xg[Wk*128:(Wk+Dk)*128].reshape(Dk, 128, C)
            s = blk + xrv[k*128:(k+1)*128][None, :, :]
            h = np.where(s > 0, s, NEG_SLOPE * s)
            e[:, Wk:Wk+Dk] = (h @ att).T
        a = np.exp(e) * mask
        for k in range(ROWS):
            Dk, Wk = int(Dc[k]), int(W[k])
            aw = a[:, Wk:Wk+Dk]
            al = aw / (aw.sum(1, keepdims=True) + 1e-16)
            blk = xg[Wk*128:(Wk+Dk)*128].reshape(Dk, 128, C)
            o = (al.T[:, :, None] * blk).sum(0) + bias_out
            out[lay["node_of"][k*128:(k+1)*128]] = np.maximum(o, 0) if relu else o
    return out


def _build_gat_neff(Dc, C):
    nc = bacc.Bacc("TRN2", target_bir_lowering=False, debug=False,
                   num_devices=NC)
    W = np.concatenate([[0], np.cumsum(Dc)])
    G = int(W[-1])
    assert G <= 512
    xgt = nc.dram_tensor("xgt", [128, G * 128], F16, kind="ExternalInput").ap()
    xpm = nc.dram_tensor("xpm", [128, NLOC], F16, kind="ExternalInput").ap()
    wlt = nc.dram_tensor("wlt", [128, C], F16, kind="ExternalInput").ap()
    wrt = nc.dram_tensor("wrt", [128, C], F16, kind="ExternalInput").ap()
    attv = nc.dram_tensor("attv", [C, 1], F16, kind="ExternalInput").ap()
    ebias = nc.dram_tensor("ebias", [C, 1], F32, kind="ExternalInput").ap()
    obias = nc.dram_tensor("obias", [C, 1], F32, kind="ExternalInput").ap()
    mask_in = nc.dram_tensor("mask", [128, G], F16, kind="ExternalInput").ap()
    ident_in = nc.dram_tensor("ident", [128, 128], F16, kind="ExternalInput").ap()
    hout = nc.dram_tensor("hout", [C, NLOC], F32, kind="ExternalOutput").ap()

    with tile.TileContext(nc) as tc, ExitStack() as ctx:
        const = ctx.enter_context(tc.tile_pool(name="c", bufs=1))
        xgp = ctx.enter_context(tc.tile_pool(name="xg", bufs=3))
        xlp = ctx.enter_context(tc.tile_pool(name="xl", bufs=3))
        shp = ctx.enter_context(tc.tile_pool(name="sh", bufs=3))
        wp = ctx.enter_context(tc.tile_pool(name="w", bufs=2))
        ap_ = ctx.enter_context(tc.tile_pool(name="al", bufs=3))
        psx = ctx.enter_context(tc.tile_pool(name="psx", bufs=3, space="PSUM"))
        pse = ctx.enter_context(tc.tile_pool(name="pse", bufs=2, space="PSUM"))
        psa = ctx.enter_context(tc.tile_pool(name="psa", bufs=3, space="PSUM"))
        outp = ctx.enter_context(tc.tile_pool(name="o", bufs=1))

        t_wlt = const.tile([128, C], F16)
        nc.sync.dma_start(t_wlt[:], wlt[:])
        t_wrt = const.tile([128, C], F16)
        nc.sync.dma_start(t_wrt[:], wrt[:])
        t_att = const.tile([C, 1], F16)
        nc.sync.dma_start(t_att[:], attv[:])
        t_eb = const.tile([C, 1], F32)
        nc.sync.dma_start(t_eb[:], ebias[:])
        t_ob = const.tile([C, 1], F32)
        nc.sync.dma_start(t_ob[:], obias[:])
        t_mask = const.tile([128, G], F16)
        nc.sync.dma_start(t_mask[:], mask_in[:])
        t_id = const.tile([128, 128], F16)
        nc.sync.dma_start(t_id[:], ident_in[:])
        t_xpm = const.tile([128, NLOC], F16)
        nc.sync.dma_start(t_xpm[:], xpm[:])

        t_out = outp.tile([C, NLOC], F32)
        SC = 4  # psum chunk: 4 j-tiles = 512 cols
        DMAX = int(Dc.max())

        # xr_T = Wr @ xpm (+ ebias), computed once: [C, NLOC] f16
        t_xr = const.tile([C, NLOC], F16)
        for i in range(NLOC // 512):
            t_pxr = psx.tile([C, 512], F32, tag="px")
            nc.tensor.matmul(t_pxr[:], t_wrt[:], t_xpm[:, bass.ts(i, 512)],
                             start=True, stop=True)
            nc.vector.tensor_scalar_add(t_xr[:, bass.ts(i, 512)], t_pxr[:],
                                        t_eb[:])

        for k in range(ROWS):
            Dk, Wk = int(Dc[k]), int(W[k])
            ncols = Dk * 128
            t_xg = xgp.tile([128, DMAX * 128], F16, tag="xg")
            nc.sync.dma_start(t_xg[:, :ncols], xgt[:, Wk*128:(Wk+Dk)*128])
            t_xls = xlp.tile([C, DMAX * 128], F16, tag="xls")
            t_ep = pse.tile([128, DMAX], F32, tag="pe")
            for c0 in range(0, Dk, SC):
                w = min(SC, Dk - c0)
                cw = w * 128
                # XLs(psum) = Wl @ xg ; evac f16
                t_px = psx.tile([C, 512], F32, tag="px")
                nc.tensor.matmul(t_px[:, :cw], t_wlt[:],
                                 t_xg[:, (c0*128):(c0*128)+cw],
                                 start=True, stop=True)
                nc.scalar.activation(t_xls[:, (c0*128):(c0*128)+cw],
                                     t_px[:, :cw],
                                     mybir.ActivationFunctionType.Copy)
                # S = XLs + xr_bcast ; H = max(S, 0.2*S)  (all f16 on DVE)
                t_se = shp.tile([C, 512], F16, tag="se")
                xrb = t_xr[:, k*128:(k+1)*128].unsqueeze(1) \
                    .broadcast_to([C, w, 128])
                nc.vector.tensor_add(
                    t_se[:, :cw].rearrange("a (b c) -> a b c", c=128),
                    t_xls[:, (c0*128):(c0*128)+cw].rearrange(
                        "a (b c) -> a b c", c=128), xrb)
                t_h0 = shp.tile([C, 512], F16, tag="h0")
                nc.vector.tensor_scalar_mul(t_h0[:, :cw], t_se[:, :cw], NEG_SLOPE)
                t_h = shp.tile([C, 512], F16, tag="h")
                nc.vector.tensor_tensor(t_h[:, :cw], t_se[:, :cw], t_h0[:, :cw],
                                        op=mybir.AluOpType.max)
                # e per 128-col tile: lhsT = H-tile, rhs = att -> psum col
                for t in range(w):
                    nc.tensor.matmul(t_ep[:, c0+t:c0+t+1],
                                     t_h[:, t*128:(t+1)*128], t_att[:],
                                     start=True, stop=True)
            # per-row softmax: a = exp(e)*mask ; alpha = a/(sum+eps)
            t_e = ap_.tile([128, DMAX], F32, tag="ee")
            nc.scalar.activation(t_e[:, :Dk], t_ep[:, :Dk],
                                 mybir.ActivationFunctionType.Exp)
            t_a = ap_.tile([128, DMAX], F32, tag="aa")
            nc.vector.tensor_mul(t_a[:, :Dk], t_e[:, :Dk], t_mask[:, Wk:Wk+Dk])
            t_s = ap_.tile([128, 1], F32, tag="ssum")
            nc.vector.tensor_reduce(t_s[:], t_a[:, :Dk],
                                    axis=mybir.AxisListType.X,
                                    op=mybir.AluOpType.add)
            t_sr = ap_.tile([128, 1], F32, tag="sr")
            nc.vector.tensor_scalar_add(t_sr[:], t_s[:], 1e-16)
            t_r = ap_.tile([128, 1], F32, tag="rr")
            nc.vector.reciprocal(t_r[:], t_sr[:])
            t_al16 = ap_.tile([128, DMAX], F16, tag="al16")
            nc.vector.tensor_scalar_mul(t_al16[:, :Dk], t_a[:, :Dk], t_r[:])
            # aggregation: aRep per-tile + W-mult (d-innermost) + reduce
            t_w = wp.tile([C, 128, DMAX], F16, tag="wv")
            for c0 in range(0, Dk, SC):
                w = min(SC, Dk - c0)
                cw = w * 128
                t_pa = psa.tile([128, 512], F32, tag="pa")
                for t in range(w):
                    nc.tensor.matmul(
                        t_pa[:, t*128:(t+1)*128],
                        t_al16[:, c0+t:c0+t+1].broadcast_to([128, 128]),
                        t_id[:], start=True, stop=True)
                wout = t_w[:, :, c0:c0+w].transpose([0, 2, 1])
                nc.vector.tensor_mul(
                    wout,
                    t_xls[:, c0*128:(c0*128)+cw].rearrange(
                        "a (b c) -> a b c", c=128),
                    t_pa[:C, :cw].rearrange("a (b c) -> a b c", c=128))
            nc.vector.tensor_reduce(t_out[:, k*128:(k+1)*128],
                                    t_w[:, :, :Dk],
                                    axis=mybir.AxisListType.X,
                                    op=mybir.AluOpType.add)
        nc.vector.tensor_scalar_add(t_out[:], t_out[:], t_ob[:])
        nc.sync.dma_start(hout[:], t_out[:])
    nc.compile()
    return nc


def _build_adj_neff():
    nc = bacc.Bacc("TRN2", target_bir_lowering=False, debug=False,
                   num_devices=NC)
    zT = nc.dram_tensor("zT", [OUT, N], BF16, kind="ExternalInput").ap()
    zTl = nc.dram_tensor("zTl", [OUT, NLOC], BF16, kind="ExternalInput").ap()
    adj = nc.dram_tensor("adj", [NLOC, N], F16, kind="ExternalOutput").ap()
    NCH = 512
    with tile.TileContext(nc) as tc, ExitStack() as ctx:
        const = ctx.enter_context(tc.tile_pool(name="c", bufs=1))
        obuf = ctx.enter_context(tc.tile_pool(name="o", bufs=3))
        psum = ctx.enter_context(tc.tile_pool(name="ps", bufs=4, space="PSUM"))
        t_z = const.tile([OUT, N], BF16)
        nc.sync.dma_start(t_z[:], zT[:])
        t_zl = const.tile([OUT, NLOC], BF16)
        nc.sync.dma_start(t_zl[:], zTl[:])
        for mt in range(NLOC // 128):
            t_o = obuf.tile([128, N], F16, tag="o")
            for cc in range(N // (2 * NCH)):
                t_ps = psum.tile([128, 2 * NCH], F32, tag="ps")
                nc.tensor.matmul(t_ps[:, :NCH], t_zl[:, bass.ts(mt, 128)],
                                 t_z[:, bass.ts(2 * cc, NCH)],
                                 start=True, stop=True)
                nc.tensor.matmul(t_ps[:, NCH:], t_zl[:, bass.ts(mt, 128)],
                                 t_z[:, bass.ts(2 * cc + 1, NCH)],
                                 start=True, stop=True)
                nc.scalar.activation(t_o[:, bass.ts(cc, 2 * NCH)], t_ps[:],
                                     mybir.ActivationFunctionType.Sigmoid)
            nc.sync.dma_start(adj[bass.ts(mt, 128), :], t_o[:])
    nc.compile()
    return nc


def _gat_layer(cores, Dc, W, G, x_feat, Wl, bl, Wr, br, att, b, relu):
    """One GAT layer. x_feat [N, 128-padded] f32. Returns out [N, C] f32."""
    C = Wl.shape[0]
    xl = x_feat @ Wl.T
    xr = x_feat @ Wr.T
    if HOST_GAT:
        h = _gat_host(cores, G, W, Dc, xl.astype(np.float16).astype(np.float32),
                      xr + (br + bl), att, bl + b, relu)
        return h
    key = ("gat", C, tuple(Dc))
    if key not in _cache:
        _cache[key] = _build_gat_neff(Dc, C)
    ncg = _cache[key]
    xT16 = np.ascontiguousarray(x_feat.T).astype(np.float16)  # [128, N]
    in_maps = []
    for lay in cores:
        in_maps.append({
            "xgt": np.ascontiguousarray(xT16[:, lay["gsrc"]]),
            "xpm": np.ascontiguousarray(xT16[:, lay["node_of"]]),
            "wlt": np.ascontiguousarray(Wl.T).astype(np.float16),
            "wrt": np.ascontiguousarray(Wr.T).astype(np.float16),
            "attv": att.reshape(C, 1).astype(np.float16),
            "ebias": (bl + br).reshape(C, 1).astype(np.float32),
            "obias": (bl + b).reshape(C, 1).astype(np.float32),
            "mask": lay["mask"],
            "ident": np.eye(128, dtype=np.float16),
        })
    res = run_bass_kernel_spmd(ncg, in_maps, list(range(NC)))
    global LAST_HW_NS
    LAST_HW_NS += _sim_ns(key, ncg)
    out = np.zeros((N, C), dtype=np.float32)
    for ci, lay in enumerate(cores):
        hT = res.results[ci]["hout"]          # [C, NLOC]
        out[lay["node_of"], :] = hT.T
    if relu:
        out = np.maximum(out, 0.0)
    return out


def kernel(x, edge_index, Wl1, bl1, Wr1, br1, att1, b1,
           Wl2, bl2, Wr2, br2, att2, b2):
    x = np.asarray(x, dtype=np.float32)
    edge_index = np.asarray(edge_index)
    Wl1, bl1, Wr1, br1, att1, b1 = [np.asarray(a, np.float32) for a in
                                    (Wl1, bl1, Wr1, br1, att1, b1)]
    Wl2, bl2, Wr2, br2, att2, b2 = [np.asarray(a, np.float32) for a in
                                    (Wl2, bl2, Wr2, br2, att2, b2)]
    global LAST_HW_NS
    LAST_HW_NS = 0
    ek = hash(edge_index.tobytes())
    if ("layout", ek) not in _cache:
        _cache[("layout", ek)] = _edge_layout(edge_index)
    cores, Dc, W, G = _cache[("layout", ek)]

    h = _gat_layer(cores, Dc, W, G, x, Wl1, bl1, Wr1, br1, att1, b1, True)
    h_pad = np.zeros((N, 128), dtype=np.float32)
    h_pad[:, :HID] = h
    out2 = _gat_layer(cores, Dc, W, G, h_pad, Wl2, bl2, Wr2, br2, att2, b2,
                      False)

    import jax, jax.numpy as jnp
    cpu = jax.devices("cpu")[0]
    with jax.default_device(cpu):
        alpha = np.asarray(jax.nn.softplus(
            jax.device_put(jnp.asarray(out2), cpu))) + 1e-6
        g = jax.random.gamma(jax.random.key(42), jnp.asarray(alpha))
        z = np.asarray(g / jnp.sum(g, axis=-1, keepdims=True),
                       dtype=np.float32)
    alpha = alpha.astype(np.float32)

    if "adj" not in _cache:
        _cache["adj"] = _build_adj_neff()
    nca = _cache["adj"]
    import ml_dtypes
    zT16 = np.ascontiguousarray(z.T).astype(ml_dtypes.bfloat16)
    in_maps = [{"zT": zT16,
                "zTl": np.ascontiguousarray(zT16[:, ci*NLOC:(ci+1)*NLOC])}
               for ci in range(NC)]
    res = run_bass_kernel_spmd(nca, in_maps, list(range(NC)))
    LAST_HW_NS += _sim_ns("adj", nca)
    adj = np.empty((N, N), dtype=np.float32)
    for ci in range(NC):
        adj[ci*NLOC:(ci+1)*NLOC, :] = res.results[ci]["adj"]
    return adj, alpha, z
